# revision 17
# baseline (speedup 1.0000x reference)
"""Trainium2 Bass kernel for nn_GCN_12979391169341 (GNN message passing).

Reference computation (N=2048 nodes, B=16, C_IN=32, C_OUT=64, E=16, K=3):
    A1 = A[1]
    conv_E_l = A1 @ conv_E
    scores = relu(conv_E_l @ conv_E.T)
    supports S = softmax(scores, axis=1)
    S_set = [I, S, 2*S@S - I]           (Chebyshev)
    x_g[b,n,k,c] = sum_m S_k[n,m] x[b,m,c]
    out[b,n,o]   = sum_{k,i} x_g[b,n,k,i] * weight[n,k,i,o] + b

Sharding: node-parallel over 8 cores (256 nodes each). Each core computes
its 256 rows of S, xg1 = S@x for its rows, all-gathers xg1 (2 chunked
collectives), then xg2' = S@xg1_all with the Chebyshev combination folded
into host-side weights: W0' = W0 - W2, W2' = 2*W2, so
out = x@W0' + xg1@W1 + (S@xg1)@W2'.

Stage H packs 8 nodes per matmul: stationary = xg slabs [96, 8*16],
moving = weight panel [96, 8*64]; the valid diagonal [16,64] blocks are
DMA'd straight from PSUM to the DRAM output.
"""

import numpy as np
import sys

sys.path.insert(0, "/opt/trn_rl_repo")

import concourse.bass as bass
import concourse.mybir as mybir
import concourse.tile as tile
from concourse.bass_utils import run_bass_kernel_spmd

N = 2048      # nodes
B = 16        # batch
CI = 32       # in channels
CO = 64       # out channels
E = 16        # conv_E dim
KCH = 3       # Chebyshev order
NCORE = 8
NL = N // NCORE          # 256 local nodes
BC = B * CI              # 512
KI = KCH * CI            # 96
DT = mybir.dt.float32
MCH = N // 128           # 16 m-chunks
HN = NL // 128           # 2 n-halves

_CACHE = {}
_DEBUG = [False]
_SPLIT_WAITS = [True]
_RUN_KWARGS = {}
_LAST_RESULT = [None]


def _build():
    nc = bass.Bass(num_devices=NCORE)
    BF = mybir.dt.bfloat16

    # ---- I/O ----
    a1t = nc.dram_tensor("a1t", [N, NL], DT, kind="ExternalInput")      # A1[nc,:].T
    et = nc.dram_tensor("et", [E, N], DT, kind="ExternalInput")         # conv_E.T
    e_nat = nc.dram_tensor("e_nat", [N, E], DT, kind="ExternalInput")   # conv_E
    xfull = nc.dram_tensor("xfull", [N, BC], BF, kind="ExternalInput")  # X[m, b*CI+i]
    xgt0 = nc.dram_tensor("xgt0", [CI, B * NL], BF, kind="ExternalInput")  # [i, b*NL+n]
    wt = nc.dram_tensor("wt", [KI, NL * CO], BF, kind="ExternalInput")  # folded W
    ident = nc.dram_tensor("ident", [128, 128], BF, kind="ExternalInput")
    outd = nc.dram_tensor("out", [B, NL, CO], BF, kind="ExternalOutput")

    # internal DRAM for the chunked collective (bf16 payload)
    xg1_own = nc.dram_tensor("xg1_own", [NL, BC], BF)
    xg1_all = [
        nc.dram_tensor(f"xg1_all{h}", [NCORE * 128, BC], BF, addr_space="Shared")
        for h in range(HN)
    ]

    dbg = {}
    if _DEBUG[0]:
        dbg["delt"] = nc.dram_tensor("delt", [E, NL], DT, kind="ExternalOutput")
        dbg["dS"] = nc.dram_tensor("dS", [HN * 128, N], mybir.dt.bfloat16, kind="ExternalOutput")
        dbg["dxg1"] = nc.dram_tensor("dxg1", [HN * 128, BC], mybir.dt.bfloat16, kind="ExternalOutput")
        dbg["dga"] = nc.dram_tensor("dga", [128, MCH * BC], mybir.dt.bfloat16, kind="ExternalOutput")
        dbg["dxgt"] = nc.dram_tensor("dxgt", [KI, NL * B], mybir.dt.bfloat16, kind="ExternalOutput")
        dbg["dst"] = nc.dram_tensor("dst", [128, MCH * NL], mybir.dt.bfloat16, kind="ExternalOutput")
        dbg["dxg2"] = nc.dram_tensor("dxg2", [128, 4 * NL], mybir.dt.bfloat16, kind="ExternalOutput")

    a1t_r = a1t.rearrange("(c p) n -> p c n", p=128)
    e_r = e_nat.rearrange("(c p) e -> p c e", p=128)
    xf_r = xfull.rearrange("(c p) n -> p c n", p=128)

    with tile.TileContext(nc) as tc:
        with (
            tc.tile_pool(name="const", bufs=1) as cpool,
            tc.tile_pool(name="stream", bufs=2) as spool,
            tc.tile_pool(name="psum", bufs=2, space="PSUM") as pp,
        ):
            # ================= input loads =================
            # small fp32 operands first (scalar/vector engines issue)
            e_sb = cpool.tile([128, MCH, E], DT)
            nc.scalar.dma_start(e_sb[:], e_r[:])
            et_sb = cpool.tile([E, N], DT)
            nc.scalar.dma_start(et_sb[:], et[:])
            id_sb = cpool.tile([128, 128], BF)
            nc.scalar.dma_start(id_sb[:], ident[:])

            # a1t chunk-wise (highest priority big tensor; feeds stage B)
            a1t_sb = cpool.tile([128, MCH, NL], DT)
            lde = [nc.sync, nc.gpsimd]
            for mc in range(MCH):
                lde[mc % 2].dma_start(a1t_sb[:, mc, :], a1t_r[:, mc, :])

            # x (needed at stage E)
            xf_sb = cpool.tile([128, MCH, BC], BF)
            for mc in range(MCH):
                lde[mc % 2].dma_start(xf_sb[:, mc, :], xf_r[:, mc, :])

            # xgt rows 0:32 (= x.T per-node layout) and weights (needed at H)
            xgt_sb = cpool.tile([KI, B * NL], BF)
            nc.scalar.dma_start(xgt_sb[0:CI, :], xgt0[:])
            wt_sb = cpool.tile([KI, NL * CO], BF)
            for i in range(4):
                lde[i % 2].dma_start(
                    wt_sb[:, 4096 * i:4096 * (i + 1)], wt[:, 4096 * i:4096 * (i + 1)]
                )

            # ============ stage B: E_lT[e, n] = sum_m E[m,e] * A1T[m,n] ============
            elt_ps = pp.tile([128, 512], DT, tag="c", name="elt_ps")[:E, :NL]
            for mc in range(MCH):
                nc.tensor.matmul(
                    elt_ps[:], e_sb[:, mc, :], a1t_sb[:, mc, :],
                    start=(mc == 0), stop=(mc == MCH - 1),
                )
            elt_sb = cpool.tile([E, NL], DT)
            nc.vector.tensor_copy(elt_sb[:], elt_ps[:])

            # ============ stage C + softmax (fp32 scores, bf16 normalized S) =======
            exps = [cpool.tile([128, N], BF, name=f"exps{h}") for h in range(HN)]
            rrecs = []
            for h in range(HN):
                relu_sb = spool.tile([128, N], DT, tag="relu", bufs=2)
                rm4 = spool.tile([128, 4], DT, tag="rm4")
                rs4 = spool.tile([128, 4], DT, tag="rs4")
                rmax = spool.tile([128, 1], DT, tag="rmax")
                nmax = spool.tile([128, 1], DT, tag="nmax")
                rsum = spool.tile([128, 1], DT, tag="rsum")
                rrec = spool.tile([128, 1], DT, tag="rrec", bufs=2)
                for mt in range(4):
                    sc_ps = pp.tile([128, 512], DT, tag="c", name="sc_ps")
                    nc.tensor.matmul(
                        sc_ps[:],
                        elt_sb[:, h * 128:(h + 1) * 128],
                        et_sb[:, mt * 512:(mt + 1) * 512],
                        start=True, stop=True,
                    )
                    nc.scalar.activation(
                        relu_sb[:, mt * 512:(mt + 1) * 512], sc_ps[:],
                        mybir.ActivationFunctionType.Relu,
                    )
                    nc.vector.tensor_reduce(
                        rm4[:, mt:mt + 1], relu_sb[:, mt * 512:(mt + 1) * 512],
                        axis=mybir.AxisListType.X, op=mybir.AluOpType.max,
                    )
                nc.vector.tensor_reduce(
                    rmax[:], rm4[:], axis=mybir.AxisListType.X,
                    op=mybir.AluOpType.max,
                )
                nc.vector.tensor_scalar_mul(nmax[:], rmax[:], -1.0)
                for mt in range(4):
                    nc.scalar.activation(
                        exps[h][:, mt * 512:(mt + 1) * 512],
                        relu_sb[:, mt * 512:(mt + 1) * 512],
                        mybir.ActivationFunctionType.Exp,
                        bias=nmax[:], accum_out=rs4[:, mt:mt + 1],
                    )
                nc.vector.tensor_reduce(
                    rsum[:], rs4[:], axis=mybir.AxisListType.X,
                    op=mybir.AluOpType.add,
                )
                nc.vector.reciprocal(rrec[:], rsum[:])
                rrecs.append(rrec)
                # normalize in 4 column chunks (vector/pool split)
                for mt in range(4):
                    eng = nc.vector if mt % 2 == 0 else nc.gpsimd
                    eng.tensor_scalar_mul(
                        exps[h][:, mt * 512:(mt + 1) * 512],
                        exps[h][:, mt * 512:(mt + 1) * 512], rrec[:],
                    )

            # ============ stage D: transpose S -> ST[m, n] (PE, bf16) =============
            st_sb = cpool.tile([128, MCH, NL], BF)
            for h in range(HN):
                for g in range(2):          # 8 m-chunks per psum bank
                    tp8 = pp.tile([128, 8, 128], BF, tag="tp", bufs=1)
                    for s in range(8):
                        nc.tensor.transpose(
                            tp8[:, s, :],
                            exps[h][:, (8 * g + s) * 128:(8 * g + s + 1) * 128],
                            id_sb[:],
                        )
                    nc.vector.tensor_copy(
                        st_sb[:, 8 * g:8 * g + 4, h * 128:(h + 1) * 128],
                        tp8[:, 0:4, :],
                    )
                    nc.scalar.copy(
                        st_sb[:, 8 * g + 4:8 * g + 8, h * 128:(h + 1) * 128],
                        tp8[:, 4:8, :],
                    )

            # ============ stage E: xg1[n, bc] = ST.T @ X; AG per half ============
            xg1_sb = []
            for h in range(HN):
                ps1 = pp.tile([128, BC], DT, tag="ee", name="ps1")
                for mc in range(MCH):
                    nc.tensor.matmul(
                        ps1[:],
                        st_sb[:, mc, h * 128:(h + 1) * 128],
                        xf_sb[:, mc, :],
                        start=(mc == 0), stop=(mc == MCH - 1),
                    )
                xsb = cpool.tile([128, BC], BF, name=f"xg1_sb{h}")
                nc.vector.tensor_copy(xsb[:], ps1[:])
                xg1_sb.append(xsb)
                nc.sync.dma_start(xg1_own[h * 128:(h + 1) * 128, :], xsb[:])
                # chunked AllGather of this half
                nc.gpsimd.collective_compute(
                    "AllGather",
                    mybir.AluOpType.bypass,
                    replica_groups=[list(range(NCORE))],
                    ins=[xg1_own[h * 128:(h + 1) * 128, :]],
                    outs=[xg1_all[h][:]],
                )
                # k1 remap: transpose xg1 -> staging sbuf -> xgt rows [CI, 2*CI)
                xgt_k1 = xgt_sb[CI:2 * CI, :].rearrange(
                    "i (n b) -> i n b", b=B
                )
                tp2 = pp.tile([128, 4, 128], BF, tag="tp2", bufs=1)
                for cc in range(4):
                    nc.tensor.transpose(
                        tp2[:, cc, :], xsb[:, cc * 128:(cc + 1) * 128], id_sb[:]
                    )
                k1s = spool.tile([128, 4, 128], BF, tag="k1s", bufs=2)
                nc.vector.tensor_copy(k1s[:, 0:2, :], tp2[:, 0:2, :])
                nc.scalar.copy(k1s[:, 2:4, :], tp2[:, 2:4, :])
                for cc in range(4):
                    for r in range(4):
                        lde[r % 2].dma_start(
                            xgt_k1[:, h * 128:(h + 1) * 128, 4 * cc + r],
                            k1s[r * 32:(r + 1) * 32, cc, :],
                        )

            # ============ stage G2: xg2'[bc, n] = XG1_ALL.T-chunks vs ST ==========
            ga_sb = cpool.tile([128, MCH, BC], BF)
            pst2 = pp.tile([128, 4, NL], DT, tag="g2", name="pst2", bufs=1)
            for h in range(HN):
                ga_r = xg1_all[h].rearrange("(c p) n -> p c n", p=128)
                for c in range(NCORE):
                    mc = 2 * c + h
                    lde[c % 2].dma_start(ga_sb[:, mc, :], ga_r[:, c, :])
                    for q in range(4):
                        # start=True zeroes the whole PSUM bank; q pairs
                        # (0,1) and (2,3) share banks, so only q0/q2 start.
                        nc.tensor.matmul(
                            pst2[:, q, :],
                            ga_sb[:, mc, q * 128:(q + 1) * 128],
                            st_sb[:, mc, :],
                            start=(h == 0 and c == 0 and q % 2 == 0),
                            stop=(h == HN - 1 and c == NCORE - 1),
                            skip_group_check=True,
                        )
            xg2t_sb = cpool.tile([128, 4, NL], BF)
            nc.vector.tensor_copy(xg2t_sb[:], pst2[:])
            xgt_k2 = xgt_sb[2 * CI:3 * CI, :].rearrange("i (n b) -> i n b", b=B)
            for q in range(4):
                for r in range(4):
                    lde[(q + r) % 2].dma_start(
                        xgt_k2[:, :, 4 * q + r],
                        xg2t_sb[r * 32:(r + 1) * 32, q, :],
                    )

            # ============ stage H: out[b,n,o] = sum_ki xgT[ki,b,n] W[ki,n,o] ======
            # 8 nodes per matmul; full [128,512] psum copied to bf16 staging,
            # then diagonal [16,64] blocks DMA'd (batched over 4 groups).
            stg = cpool.tile([128, NL // 8, 512], BF)
            outd_v = outd.rearrange("b (g tp j) o -> b g tp j o", g=8, tp=4, j=8)
            for t in range(NL // 8):
                pso = pp.tile([128, 512], DT, tag="ee", name="pso")
                nc.tensor.matmul(
                    pso[:],
                    xgt_sb[:, t * 128:(t + 1) * 128],
                    wt_sb[:, t * 512:(t + 1) * 512],
                    start=True, stop=True,
                )
                if t % 2 == 1:
                    nc.scalar.copy(stg[:, t, :], pso[:])
                else:
                    nc.vector.tensor_copy(stg[:, t, :], pso[:])
                if t % 4 == 3:
                    g = t // 4
                    for j in range(8):
                        lde[j % 2].dma_start(
                            outd_v[:, g, :, j, :],
                            stg[16 * j:16 * j + B, 4 * g:4 * (g + 1),
                                64 * j:64 * (j + 1)],
                        )

            if _DEBUG[0]:
                nc.sync.dma_start(dbg["delt"][:], elt_sb[:])
                for h in range(HN):
                    nc.sync.dma_start(dbg["dS"][h * 128:(h + 1) * 128, :], exps[h][:])
                    nc.sync.dma_start(dbg["dxg1"][h * 128:(h + 1) * 128, :], xg1_sb[h][:])
                nc.sync.dma_start(dbg["dga"][:], ga_sb[:, :].rearrange("p c n -> p (c n)"))
                nc.sync.dma_start(dbg["dxgt"][:], xgt_sb[:])
                nc.sync.dma_start(dbg["dst"][:], st_sb[:, :].rearrange("p c n -> p (c n)"))
                nc.sync.dma_start(dbg["dxg2"][:], xg2t_sb[:, :].rearrange("p c n -> p (c n)"))

    if _SPLIT_WAITS[0]:
        _split_matmul_waits(nc)
    return nc


def _split_matmul_waits(nc):
    """walrus encodes at most one sync-wait per TPB instruction (the EVENTS
    struct has a single wait slot); hoist extra waits onto preceding
    same-engine no-ops."""
    f = nc.m.functions[0]
    for blk in f.blocks:
        insts = blk.instructions
        out = []
        changed = False
        for inst in insts:
            si = inst.sync_info
            if (
                si is not None and si.on_wait and len(si.on_wait) > 1
            ):
                waits = list(si.on_wait)
                for k, w in enumerate(waits[:-1]):
                    nop = mybir.InstNoOp(
                        name=f"{inst.name}-wsplit-{k}",
                        engine=inst.engine,
                        sync_info=mybir.SyncInfo(on_wait=[w], on_update=[]),
                    )
                    out.append(nop)
                inst.sync_info = mybir.SyncInfo(
                    on_wait=[waits[-1]], on_update=list(si.on_update or [])
                )
                changed = True
            out.append(inst)
        if changed:
            blk.instructions = out


def kernel(A, x, conv_E, weight, b):
    A = np.asarray(A, dtype=np.float32)
    x = np.asarray(x, dtype=np.float32)
    conv_E = np.asarray(conv_E, dtype=np.float32)
    weight = np.asarray(weight, dtype=np.float32)
    b = np.asarray(b, dtype=np.float32)

    if "nc" not in _CACHE:
        _CACHE["nc"] = _build()
    nc = _CACHE["nc"]

    import ml_dtypes
    BF = ml_dtypes.bfloat16

    X = np.ascontiguousarray(x.transpose(1, 0, 2).reshape(N, BC))
    X_bf = X.astype(BF)
    ET = np.ascontiguousarray(conv_E.T)
    IDENT = np.eye(128, dtype=BF)
    A1 = A[1]
    # fold Chebyshev combination into weights:
    # out = x@(W0 - W2) + xg1@W1 + (S@xg1)@(2*W2)
    wf = np.empty_like(weight)
    wf[:, 0] = weight[:, 0] - weight[:, 2]
    wf[:, 1] = weight[:, 1]
    wf[:, 2] = 2.0 * weight[:, 2]

    in_maps = []
    for c in range(NCORE):
        sl = slice(c * NL, (c + 1) * NL)
        a1t_c = np.ascontiguousarray(A1[sl, :].T)                       # [N, NL]
        xgt0_c = np.ascontiguousarray(
            X[sl].reshape(NL, B, CI).transpose(2, 0, 1).reshape(CI, NL * B)
        ).astype(BF)
        wt_c = np.ascontiguousarray(
            wf[sl].transpose(1, 2, 0, 3).reshape(KI, NL * CO)
        ).astype(BF)
        in_maps.append({
            "a1t": a1t_c, "et": ET, "e_nat": conv_E, "xfull": X_bf,
            "xgt0": xgt0_c, "wt": wt_c, "ident": IDENT,
        })

    res = run_bass_kernel_spmd(nc, in_maps, core_ids=list(range(NCORE)), **_RUN_KWARGS)
    _LAST_RESULT[0] = res
    full = np.concatenate(
        [np.asarray(res.results[c]["out"], dtype=np.float32) for c in range(NCORE)],
        axis=1,
    )
    return (full + b[None, None, :]).astype(np.float32)


# revision 20
# speedup vs baseline: 2.6803x; 2.6803x over previous
"""Trainium2 Bass kernel for nn_GCN_12979391169341 (GNN message passing).

Reference computation (N=2048 nodes, B=16, C_IN=32, C_OUT=64, E=16, K=3):
    A1 = A[1]
    conv_E_l = A1 @ conv_E
    scores = relu(conv_E_l @ conv_E.T)
    supports S = softmax(scores, axis=1)
    S_set = [I, S, 2*S@S - I]           (Chebyshev)
    x_g[b,n,k,c] = sum_m S_k[n,m] x[b,m,c]
    out[b,n,o]   = sum_{k,i} x_g[b,n,k,i] * weight[n,k,i,o] + b

Sharding: node-parallel over 8 cores (256 nodes each). Each core computes
its 256 rows of S, xg1 = S@x for its rows, all-gathers xg1 (2 chunked
collectives), then xg2' = S@xg1_all with the Chebyshev combination folded
into host-side weights: W0' = W0 - W2, W2' = 2*W2, so
out = x@W0' + xg1@W1 + (S@xg1)@W2'.

Stage H packs 8 nodes per matmul: stationary = xg slabs [96, 8*16],
moving = weight panel [96, 8*64]; the valid diagonal [16,64] blocks are
DMA'd straight from PSUM to the DRAM output.
"""

import numpy as np
import sys

sys.path.insert(0, "/opt/trn_rl_repo")

import concourse.bass as bass
import concourse.mybir as mybir
import concourse.tile as tile
from concourse.bass_utils import run_bass_kernel_spmd

N = 2048      # nodes
B = 16        # batch
CI = 32       # in channels
CO = 64       # out channels
E = 16        # conv_E dim
KCH = 3       # Chebyshev order
NCORE = 8
NL = N // NCORE          # 256 local nodes
BC = B * CI              # 512
KI = KCH * CI            # 96
DT = mybir.dt.float32
MCH = N // 128           # 16 m-chunks
HN = NL // 128           # 2 n-halves

_CACHE = {}
_DEBUG = [False]
_SPLIT_WAITS = [True]
_RUN_KWARGS = {}
_LAST_RESULT = [None]


def _build():
    nc = bass.Bass(num_devices=NCORE)
    BF = mybir.dt.bfloat16

    # ---- I/O ----
    a1t = nc.dram_tensor("a1t", [N, NL], DT, kind="ExternalInput")      # A1[nc,:].T
    et = nc.dram_tensor("et", [E, N], DT, kind="ExternalInput")         # conv_E.T
    e_nat = nc.dram_tensor("e_nat", [N, E], DT, kind="ExternalInput")   # conv_E
    xfull = nc.dram_tensor("xfull", [N, BC], BF, kind="ExternalInput")  # X[m, b*CI+i]
    xgt0 = nc.dram_tensor("xgt0", [CI, B * NL], BF, kind="ExternalInput")  # [i, b*NL+n]
    wt = nc.dram_tensor("wt", [KI, NL * CO], BF, kind="ExternalInput")  # folded W
    ident = nc.dram_tensor("ident", [128, 128], BF, kind="ExternalInput")
    outd = nc.dram_tensor("out", [B, NL, CO], BF, kind="ExternalOutput")

    # internal DRAM for the chunked collective (bf16 payload)
    xg1_own = nc.dram_tensor("xg1_own", [NL, BC], BF)
    xg1_all = [
        nc.dram_tensor(f"xg1_all{h}", [NCORE * 128, BC], BF, addr_space="Shared")
        for h in range(HN)
    ]

    dbg = {}
    if _DEBUG[0]:
        dbg["delt"] = nc.dram_tensor("delt", [E, NL], DT, kind="ExternalOutput")
        dbg["dS"] = nc.dram_tensor("dS", [HN * 128, N], mybir.dt.bfloat16, kind="ExternalOutput")
        dbg["dxg1"] = nc.dram_tensor("dxg1", [HN * 128, BC], mybir.dt.bfloat16, kind="ExternalOutput")
        dbg["dga"] = nc.dram_tensor("dga", [128, MCH * BC], mybir.dt.bfloat16, kind="ExternalOutput")
        dbg["dxgt"] = nc.dram_tensor("dxgt", [KI, NL * B], mybir.dt.bfloat16, kind="ExternalOutput")
        dbg["dst"] = nc.dram_tensor("dst", [128, MCH * NL], mybir.dt.bfloat16, kind="ExternalOutput")

    a1t_r = a1t.rearrange("(c p) n -> p c n", p=128)
    e_r = e_nat.rearrange("(c p) e -> p c e", p=128)
    xf_r = xfull.rearrange("(c p) n -> p c n", p=128)

    with tile.TileContext(nc) as tc:
        with (
            tc.tile_pool(name="const", bufs=1) as cpool,
            tc.tile_pool(name="stream", bufs=2) as spool,
            tc.tile_pool(name="psum", bufs=2, space="PSUM") as pp,
        ):
            # ================= input loads =================
            # small fp32 operands first (scalar/vector engines issue)
            e_sb = cpool.tile([128, MCH, E], DT)
            nc.scalar.dma_start(e_sb[:], e_r[:])
            et_sb = cpool.tile([E, N], DT)
            nc.scalar.dma_start(et_sb[:], et[:])
            id_sb = cpool.tile([128, 128], BF)
            nc.scalar.dma_start(id_sb[:], ident[:])

            # a1t chunk-wise (highest priority big tensor; feeds stage B)
            a1t_sb = cpool.tile([128, MCH, NL], DT)
            lde = [nc.sync, nc.gpsimd]
            for mc in range(MCH):
                lde[mc % 2].dma_start(a1t_sb[:, mc, :], a1t_r[:, mc, :])

            # x (needed at stage E)
            xf_sb = cpool.tile([128, MCH, BC], BF)
            for mc in range(MCH):
                lde[mc % 2].dma_start(xf_sb[:, mc, :], xf_r[:, mc, :])

            # xgt rows 0:32 (= x.T per-node layout) and weights (needed at H)
            xgt_sb = cpool.tile([KI, B * NL], BF)
            nc.scalar.dma_start(xgt_sb[0:CI, :], xgt0[:])
            wt_sb = cpool.tile([KI, NL * CO], BF)
            for i in range(4):
                lde[i % 2].dma_start(
                    wt_sb[:, 4096 * i:4096 * (i + 1)], wt[:, 4096 * i:4096 * (i + 1)]
                )

            # ============ stage B: E_lT[e, n] = sum_m E[m,e] * A1T[m,n] ============
            elt_ps = pp.tile([128, 512], DT, tag="c", name="elt_ps")[:E, :NL]
            for mc in range(MCH):
                nc.tensor.matmul(
                    elt_ps[:], e_sb[:, mc, :], a1t_sb[:, mc, :],
                    start=(mc == 0), stop=(mc == MCH - 1),
                )
            elt_sb = cpool.tile([E, NL], DT)
            nc.vector.tensor_copy(elt_sb[:], elt_ps[:])

            # ============ stage C + softmax (fp32 scores, bf16 normalized S) =======
            exps = [cpool.tile([128, N], BF, name=f"exps{h}") for h in range(HN)]
            rrecs = []
            for h in range(HN):
                relu_sb = spool.tile([128, N], DT, tag="relu", bufs=2)
                rm4 = spool.tile([128, 4], DT, tag="rm4")
                rs4 = spool.tile([128, 4], DT, tag="rs4")
                rmax = spool.tile([128, 1], DT, tag="rmax")
                nmax = spool.tile([128, 1], DT, tag="nmax")
                rsum = spool.tile([128, 1], DT, tag="rsum")
                rrec = spool.tile([128, 1], DT, tag="rrec", bufs=2)
                for mt in range(4):
                    sc_ps = pp.tile([128, 512], DT, tag="c", name="sc_ps")
                    nc.tensor.matmul(
                        sc_ps[:],
                        elt_sb[:, h * 128:(h + 1) * 128],
                        et_sb[:, mt * 512:(mt + 1) * 512],
                        start=True, stop=True,
                    )
                    nc.scalar.activation(
                        relu_sb[:, mt * 512:(mt + 1) * 512], sc_ps[:],
                        mybir.ActivationFunctionType.Relu,
                    )
                    nc.vector.tensor_reduce(
                        rm4[:, mt:mt + 1], relu_sb[:, mt * 512:(mt + 1) * 512],
                        axis=mybir.AxisListType.X, op=mybir.AluOpType.max,
                    )
                nc.vector.tensor_reduce(
                    rmax[:], rm4[:], axis=mybir.AxisListType.X,
                    op=mybir.AluOpType.max,
                )
                nc.vector.tensor_scalar_mul(nmax[:], rmax[:], -1.0)
                for mt in range(4):
                    nc.scalar.activation(
                        exps[h][:, mt * 512:(mt + 1) * 512],
                        relu_sb[:, mt * 512:(mt + 1) * 512],
                        mybir.ActivationFunctionType.Exp,
                        bias=nmax[:], accum_out=rs4[:, mt:mt + 1],
                    )
                nc.vector.tensor_reduce(
                    rsum[:], rs4[:], axis=mybir.AxisListType.X,
                    op=mybir.AluOpType.add,
                )
                nc.vector.reciprocal(rrec[:], rsum[:])
                rrecs.append(rrec)
                # normalize in 4 column chunks (vector/pool split)
                for mt in range(4):
                    eng = nc.vector if mt % 2 == 0 else nc.gpsimd
                    eng.tensor_scalar_mul(
                        exps[h][:, mt * 512:(mt + 1) * 512],
                        exps[h][:, mt * 512:(mt + 1) * 512], rrec[:],
                    )

            # ============ stage D: transpose S -> ST[m, n] (PE, bf16) =============
            st_sb = cpool.tile([128, MCH, NL], BF)
            for h in range(HN):
                for g in range(2):          # 8 m-chunks per psum bank
                    tp8 = pp.tile([128, 8, 128], BF, tag="tp", bufs=1)
                    for s in range(8):
                        nc.tensor.transpose(
                            tp8[:, s, :],
                            exps[h][:, (8 * g + s) * 128:(8 * g + s + 1) * 128],
                            id_sb[:],
                        )
                    nc.vector.tensor_copy(
                        st_sb[:, 8 * g:8 * g + 4, h * 128:(h + 1) * 128],
                        tp8[:, 0:4, :],
                    )
                    nc.scalar.copy(
                        st_sb[:, 8 * g + 4:8 * g + 8, h * 128:(h + 1) * 128],
                        tp8[:, 4:8, :],
                    )

            # ============ stage E: xg1[n, bc] = ST.T @ X; AG per half ============
            xg1_sb = []
            for h in range(HN):
                ps1 = pp.tile([128, BC], DT, tag="ee", name="ps1")
                for mc in range(MCH):
                    nc.tensor.matmul(
                        ps1[:],
                        st_sb[:, mc, h * 128:(h + 1) * 128],
                        xf_sb[:, mc, :],
                        start=(mc == 0), stop=(mc == MCH - 1),
                    )
                xsb = cpool.tile([128, BC], BF, name=f"xg1_sb{h}")
                nc.vector.tensor_copy(xsb[:], ps1[:])
                xg1_sb.append(xsb)
                nc.sync.dma_start(xg1_own[h * 128:(h + 1) * 128, :], xsb[:])
                # chunked AllGather of this half
                nc.gpsimd.collective_compute(
                    "AllGather",
                    mybir.AluOpType.bypass,
                    replica_groups=[list(range(NCORE))],
                    ins=[xg1_own[h * 128:(h + 1) * 128, :]],
                    outs=[xg1_all[h][:]],
                )
                # k1 remap: transpose xg1, then strided engine copies
                # (psum -> xgt directly; b-interleaved dst is DMA-hostile)
                xgt_k1 = xgt_sb[CI:2 * CI, :].rearrange(
                    "i (n b) -> i n b", b=B
                )
                tp2 = pp.tile([128, 4, 128], BF, tag="tp2", bufs=1)
                for cc in range(4):
                    nc.tensor.transpose(
                        tp2[:, cc, :], xsb[:, cc * 128:(cc + 1) * 128], id_sb[:]
                    )
                for cc in range(4):
                    for r in range(4):
                        eng = nc.vector if (cc + r) % 2 == 0 else nc.scalar
                        dst = xgt_k1[:, h * 128:(h + 1) * 128, 4 * cc + r]
                        src = tp2[r * 32:(r + 1) * 32, cc, :]
                        if (cc + r) % 2 == 0:
                            nc.vector.tensor_copy(dst, src)
                        else:
                            nc.scalar.copy(dst, src)

            # ============ stage G2: xg2'[bc, n] = XG1_ALL.T-chunks vs ST ==========
            ga_sb = cpool.tile([128, MCH, BC], BF)
            pst2 = pp.tile([128, 4, NL], DT, tag="g2", name="pst2", bufs=1)
            for h in range(HN):
                ga_r = xg1_all[h].rearrange("(c p) n -> p c n", p=128)
                for c in range(NCORE):
                    mc = 2 * c + h
                    lde[c % 2].dma_start(ga_sb[:, mc, :], ga_r[:, c, :])
                    for q in range(4):
                        # start=True zeroes the whole PSUM bank; q pairs
                        # (0,1) and (2,3) share banks, so only q0/q2 start.
                        nc.tensor.matmul(
                            pst2[:, q, :],
                            ga_sb[:, mc, q * 128:(q + 1) * 128],
                            st_sb[:, mc, :],
                            start=(h == 0 and c == 0 and q % 2 == 0),
                            stop=(h == HN - 1 and c == NCORE - 1),
                            skip_group_check=True,
                        )
            xgt_k2 = xgt_sb[2 * CI:3 * CI, :].rearrange("i (n b) -> i n b", b=B)
            for q in range(4):
                for r in range(4):
                    dst = xgt_k2[:, :, 4 * q + r]
                    src = pst2[r * 32:(r + 1) * 32, q, :]
                    if (q + r) % 2 == 0:
                        nc.vector.tensor_copy(dst, src)
                    else:
                        nc.scalar.copy(dst, src)

            # ============ stage H: out[b,n,o] = sum_ki xgT[ki,b,n] W[ki,n,o] ======
            # 8 nodes per matmul; full [128,512] psum copied to bf16 staging,
            # then diagonal [16,64] blocks DMA'd (batched over 4 groups).
            stg = cpool.tile([128, NL // 8, 512], BF)
            outd_v = outd.rearrange("b (g tp j) o -> b g tp j o", g=8, tp=4, j=8)
            for t in range(NL // 8):
                pso = pp.tile([128, 512], DT, tag="ee", name="pso")
                nc.tensor.matmul(
                    pso[:],
                    xgt_sb[:, t * 128:(t + 1) * 128],
                    wt_sb[:, t * 512:(t + 1) * 512],
                    start=True, stop=True,
                )
                if t % 2 == 1:
                    nc.scalar.copy(stg[:, t, :], pso[:])
                else:
                    nc.vector.tensor_copy(stg[:, t, :], pso[:])
                if t % 4 == 3:
                    g = t // 4
                    for j in range(8):
                        lde[j % 2].dma_start(
                            outd_v[:, g, :, j, :],
                            stg[16 * j:16 * j + B, 4 * g:4 * (g + 1),
                                64 * j:64 * (j + 1)],
                        )

            if _DEBUG[0]:
                nc.sync.dma_start(dbg["delt"][:], elt_sb[:])
                for h in range(HN):
                    nc.sync.dma_start(dbg["dS"][h * 128:(h + 1) * 128, :], exps[h][:])
                    nc.sync.dma_start(dbg["dxg1"][h * 128:(h + 1) * 128, :], xg1_sb[h][:])
                nc.sync.dma_start(dbg["dga"][:], ga_sb[:, :].rearrange("p c n -> p (c n)"))
                nc.sync.dma_start(dbg["dxgt"][:], xgt_sb[:])
                nc.sync.dma_start(dbg["dst"][:], st_sb[:, :].rearrange("p c n -> p (c n)"))

    if _SPLIT_WAITS[0]:
        _split_matmul_waits(nc)
    return nc


def _split_matmul_waits(nc):
    """walrus encodes at most one sync-wait per TPB instruction (the EVENTS
    struct has a single wait slot); hoist extra waits onto preceding
    same-engine no-ops."""
    f = nc.m.functions[0]
    for blk in f.blocks:
        insts = blk.instructions
        out = []
        changed = False
        for inst in insts:
            si = inst.sync_info
            if (
                si is not None and si.on_wait and len(si.on_wait) > 1
            ):
                waits = list(si.on_wait)
                for k, w in enumerate(waits[:-1]):
                    nop = mybir.InstNoOp(
                        name=f"{inst.name}-wsplit-{k}",
                        engine=inst.engine,
                        sync_info=mybir.SyncInfo(on_wait=[w], on_update=[]),
                    )
                    out.append(nop)
                inst.sync_info = mybir.SyncInfo(
                    on_wait=[waits[-1]], on_update=list(si.on_update or [])
                )
                changed = True
            out.append(inst)
        if changed:
            blk.instructions = out


def kernel(A, x, conv_E, weight, b):
    A = np.asarray(A, dtype=np.float32)
    x = np.asarray(x, dtype=np.float32)
    conv_E = np.asarray(conv_E, dtype=np.float32)
    weight = np.asarray(weight, dtype=np.float32)
    b = np.asarray(b, dtype=np.float32)

    if "nc" not in _CACHE:
        _CACHE["nc"] = _build()
    nc = _CACHE["nc"]

    import ml_dtypes
    BF = ml_dtypes.bfloat16

    X = np.ascontiguousarray(x.transpose(1, 0, 2).reshape(N, BC))
    X_bf = X.astype(BF)
    ET = np.ascontiguousarray(conv_E.T)
    IDENT = np.eye(128, dtype=BF)
    A1 = A[1]
    # fold Chebyshev combination into weights:
    # out = x@(W0 - W2) + xg1@W1 + (S@xg1)@(2*W2)
    wf = np.empty_like(weight)
    wf[:, 0] = weight[:, 0] - weight[:, 2]
    wf[:, 1] = weight[:, 1]
    wf[:, 2] = 2.0 * weight[:, 2]

    in_maps = []
    for c in range(NCORE):
        sl = slice(c * NL, (c + 1) * NL)
        a1t_c = np.ascontiguousarray(A1[sl, :].T)                       # [N, NL]
        xgt0_c = np.ascontiguousarray(
            X[sl].reshape(NL, B, CI).transpose(2, 0, 1).reshape(CI, NL * B)
        ).astype(BF)
        wt_c = np.ascontiguousarray(
            wf[sl].transpose(1, 2, 0, 3).reshape(KI, NL * CO)
        ).astype(BF)
        in_maps.append({
            "a1t": a1t_c, "et": ET, "e_nat": conv_E, "xfull": X_bf,
            "xgt0": xgt0_c, "wt": wt_c, "ident": IDENT,
        })

    res = run_bass_kernel_spmd(nc, in_maps, core_ids=list(range(NCORE)), **_RUN_KWARGS)
    _LAST_RESULT[0] = res
    full = np.concatenate(
        [np.asarray(res.results[c]["out"], dtype=np.float32) for c in range(NCORE)],
        axis=1,
    )
    return (full + b[None, None, :]).astype(np.float32)


# revision 25
# speedup vs baseline: 3.1279x; 1.1670x over previous
"""Trainium2 Bass kernel for nn_GCN_12979391169341 (GNN message passing).

Reference computation (N=2048 nodes, B=16, C_IN=32, C_OUT=64, E=16, K=3):
    A1 = A[1]
    conv_E_l = A1 @ conv_E
    scores = relu(conv_E_l @ conv_E.T)
    supports S = softmax(scores, axis=1)
    S_set = [I, S, 2*S@S - I]           (Chebyshev)
    x_g[b,n,k,c] = sum_m S_k[n,m] x[b,m,c]
    out[b,n,o]   = sum_{k,i} x_g[b,n,k,i] * weight[n,k,i,o] + b

Sharding: node-parallel over 8 cores (256 nodes each). Each core computes
its 256 rows of S (softmax normalization folded into the S-transpose via a
diag(1/rowsum) stationary), xg1 = S@x for its rows, one AllGather of xg1,
then xg2' = S@xg1_all with the Chebyshev combination folded into host-side
weights: W0' = W0 - W2, W2' = 2*W2, so
out = x@W0' + xg1@W1 + (S@xg1)@W2'.

xgt layout [ki, t*128 + b*8 + j] (node n = 8t + j): each stage-H stationary
slab xgt[:, t*128:(t+1)*128] is contiguous; per-node weight panels stream
as the moving operand; PSUM partition p = b*8+j, so the valid diagonal
blocks are (p%8 == moving_col/64).
"""

import numpy as np
import sys

sys.path.insert(0, "/opt/trn_rl_repo")

import concourse.bass as bass
import concourse.mybir as mybir
import concourse.tile as tile
from concourse.bass_utils import run_bass_kernel_spmd

N = 2048      # nodes
B = 16        # batch
CI = 32       # in channels
CO = 64       # out channels
E = 16        # conv_E dim
KCH = 3       # Chebyshev order
NCORE = 8
NL = N // NCORE          # 256 local nodes
BC = B * CI              # 512
KI = KCH * CI            # 96
DT = mybir.dt.float32
MCH = N // 128           # 16 m-chunks
HN = NL // 128           # 2 n-halves
NT = NL // 8             # 32 H-slabs

_CACHE = {}
_DEBUG = [False]
_SPLIT_WAITS = [True]
_RUN_KWARGS = {}
_LAST_RESULT = [None]


def _build():
    nc = bass.Bass(num_devices=NCORE)
    BF = mybir.dt.bfloat16

    # ---- I/O ----
    a1t = nc.dram_tensor("a1t", [N, NL], DT, kind="ExternalInput")      # A1[nc,:].T
    et = nc.dram_tensor("et", [E, N], DT, kind="ExternalInput")         # conv_E.T
    e_arr = nc.dram_tensor("e_arr", [128, MCH * E], DT, kind="ExternalInput")
    xfull = nc.dram_tensor("xfull", [N, BC], BF, kind="ExternalInput")  # X[m, b*CI+i]
    xgt0 = nc.dram_tensor("xgt0", [CI, NL * B], BF, kind="ExternalInput")
    wt = nc.dram_tensor("wt", [KI, NL * CO], BF, kind="ExternalInput")  # folded W
    ident = nc.dram_tensor("ident", [128, 128], BF, kind="ExternalInput")
    outd = nc.dram_tensor("out", [B, NL, CO], BF, kind="ExternalOutput")

    # internal DRAM for the collective (bf16 payload)
    xg1_own = nc.dram_tensor("xg1_own", [NL, BC], BF)
    xg1_all = nc.dram_tensor("xg1_all", [N, BC], BF, addr_space="Shared")

    dbg = {}
    if _DEBUG[0]:
        dbg["delt"] = nc.dram_tensor("delt", [E, NL], DT, kind="ExternalOutput")
        dbg["dS"] = nc.dram_tensor("dS", [HN * 128, N], BF, kind="ExternalOutput")
        dbg["dxg1"] = nc.dram_tensor("dxg1", [HN * 128, BC], BF, kind="ExternalOutput")
        dbg["dga"] = nc.dram_tensor("dga", [128, MCH * BC], BF, kind="ExternalOutput")
        dbg["dxgt"] = nc.dram_tensor("dxgt", [KI, NL * B], BF, kind="ExternalOutput")
        dbg["dst"] = nc.dram_tensor("dst", [128, MCH * NL], BF, kind="ExternalOutput")

    a1t_r = a1t.rearrange("(c p) n -> p c n", p=128)
    xf_r = xfull.rearrange("(c p) n -> p c n", p=128)

    with tile.TileContext(nc) as tc:
        with (
            tc.tile_pool(name="const", bufs=1) as cpool,
            tc.tile_pool(name="stream", bufs=2) as spool,
            tc.tile_pool(name="psum", bufs=2, space="PSUM") as pp,
        ):
            # ================= input loads =================
            e_sb = cpool.tile([128, MCH, E], DT)
            nc.scalar.dma_start(e_sb[:], e_arr.rearrange("p (c e) -> p c e", e=E))
            et_sb = cpool.tile([E, N], DT)
            nc.scalar.dma_start(et_sb[:], et[:])
            id_sb = cpool.tile([128, 128], BF)
            nc.scalar.dma_start(id_sb[:], ident[:])

            # a1t chunk-wise (highest priority big tensor; feeds stage B)
            a1t_sb = cpool.tile([128, MCH, NL], DT)
            lde = [nc.sync, nc.gpsimd]
            for mc in range(MCH):
                lde[mc % 2].dma_start(a1t_sb[:, mc, :], a1t_r[:, mc, :])

            # x (needed at stage E)
            xf_sb = cpool.tile([128, MCH, BC], BF)
            for mc in range(MCH):
                lde[mc % 2].dma_start(xf_sb[:, mc, :], xf_r[:, mc, :])

            # xgt rows 0:32 (x in [i, t*128+b*8+j] layout) and folded weights
            xgt_sb = cpool.tile([KI, NL * B], BF)
            nc.scalar.dma_start(xgt_sb[0:CI, :], xgt0[:])
            wt_sb = cpool.tile([KI, NL * CO], BF)
            for i in range(4):
                lde[i % 2].dma_start(
                    wt_sb[:, 4096 * i:4096 * (i + 1)], wt[:, 4096 * i:4096 * (i + 1)]
                )

            # ============ stage B: E_lT[e, n] = sum_m E[m,e] * A1T[m,n] ============
            elt_ps = pp.tile([128, 512], DT, tag="c", name="elt_ps")[:E, :NL]
            for mc in range(MCH):
                nc.tensor.matmul(
                    elt_ps[:], e_sb[:, mc, :], a1t_sb[:, mc, :],
                    start=(mc == 0), stop=(mc == MCH - 1),
                )
            elt_sb = cpool.tile([E, NL], DT)
            nc.vector.tensor_copy(elt_sb[:], elt_ps[:])

            # ============ stage C + softmax (fp32 scores, bf16 normalized S) ======
            # Two exp passes: pass 1 only accumulates rowsum; pass 2 uses
            # bias = -(rowmax + ln(rowsum)) so exps comes out NORMALIZED.
            exps = [cpool.tile([128, N], BF, name=f"exps{h}") for h in range(HN)]
            for h in range(HN):
                relu_sb = spool.tile([128, N], DT, tag="relu", bufs=2)
                rm4 = spool.tile([128, 4], DT, tag="rm4")
                rs4 = spool.tile([128, 4], DT, tag="rs4")
                rmax = spool.tile([128, 1], DT, tag="rmax")
                nmax = spool.tile([128, 1], DT, tag="nmax")
                rsum = spool.tile([128, 1], DT, tag="rsum")
                lnr = spool.tile([128, 1], DT, tag="lnr")
                nbias = spool.tile([128, 1], DT, tag="nbias", bufs=2)
                for mt in range(4):
                    sc_ps = pp.tile([128, 512], DT, tag="c", name="sc_ps")
                    nc.tensor.matmul(
                        sc_ps[:],
                        elt_sb[:, h * 128:(h + 1) * 128],
                        et_sb[:, mt * 512:(mt + 1) * 512],
                        start=True, stop=True,
                    )
                    nc.scalar.activation(
                        relu_sb[:, mt * 512:(mt + 1) * 512], sc_ps[:],
                        mybir.ActivationFunctionType.Relu,
                    )
                    nc.vector.tensor_reduce(
                        rm4[:, mt:mt + 1], relu_sb[:, mt * 512:(mt + 1) * 512],
                        axis=mybir.AxisListType.X, op=mybir.AluOpType.max,
                    )
                nc.vector.tensor_reduce(
                    rmax[:], rm4[:], axis=mybir.AxisListType.X,
                    op=mybir.AluOpType.max,
                )
                nc.vector.tensor_scalar_mul(nmax[:], rmax[:], -1.0)
                for mt in range(4):
                    nc.scalar.activation(
                        exps[h][:, mt * 512:(mt + 1) * 512],
                        relu_sb[:, mt * 512:(mt + 1) * 512],
                        mybir.ActivationFunctionType.Exp,
                        bias=nmax[:], accum_out=rs4[:, mt:mt + 1],
                    )
                nc.vector.tensor_reduce(
                    rsum[:], rs4[:], axis=mybir.AxisListType.X,
                    op=mybir.AluOpType.add,
                )
                nc.scalar.activation(
                    lnr[:], rsum[:], mybir.ActivationFunctionType.Ln,
                )
                nc.vector.tensor_tensor(
                    nbias[:], lnr[:], rmax[:], op=mybir.AluOpType.add,
                )
                nc.vector.tensor_scalar_mul(nbias[:], nbias[:], -1.0)
                for mt in range(4):
                    nc.scalar.activation(
                        exps[h][:, mt * 512:(mt + 1) * 512],
                        relu_sb[:, mt * 512:(mt + 1) * 512],
                        mybir.ActivationFunctionType.Exp,
                        bias=nbias[:],
                    )

            # ===== stage D: ST[m, n] = S^T (PE transposes, bf16) ====
            st_sb = cpool.tile([128, MCH, NL], BF)
            for h in range(HN):
                for g in range(2):          # 8 m-chunks per psum bank
                    tp8 = pp.tile([128, 8, 128], BF, tag="tp", bufs=1)
                    for s in range(8):
                        nc.tensor.transpose(
                            tp8[:, s, :],
                            exps[h][:, (8 * g + s) * 128:(8 * g + s + 1) * 128],
                            id_sb[:],
                        )
                    nc.vector.tensor_copy(
                        st_sb[:, 8 * g:8 * g + 4, h * 128:(h + 1) * 128],
                        tp8[:, 0:4, :],
                    )
                    nc.scalar.copy(
                        st_sb[:, 8 * g + 4:8 * g + 8, h * 128:(h + 1) * 128],
                        tp8[:, 4:8, :],
                    )

            # ============ stage E: xg1[n, bc] = ST.T @ X ============
            xg1s = cpool.tile([128, HN, BC], BF)
            xgt_k1 = xgt_sb[CI:2 * CI, :].rearrange(
                "i (t b j) -> i t b j", b=B, j=8
            )
            for h in range(HN):
                ps1 = pp.tile([128, BC], DT, tag="ee", name="ps1")
                for mc in range(MCH):
                    nc.tensor.matmul(
                        ps1[:],
                        st_sb[:, mc, h * 128:(h + 1) * 128],
                        xf_sb[:, mc, :],
                        start=(mc == 0), stop=(mc == MCH - 1),
                    )
                xsb = xg1s[:, h, :]
                nc.vector.tensor_copy(xsb, ps1[:])
                # k1 remap: transpose xg1 (already normalized), engine copies
                # into xgt rows [CI, 2*CI) with 8-elem contiguous runs
                tp2 = pp.tile([128, 4, 128], BF, tag="tp2", bufs=1)
                for cc in range(4):
                    nc.tensor.transpose(
                        tp2[:, cc, :], xsb[:, cc * 128:(cc + 1) * 128], id_sb[:]
                    )
                for cc in range(4):
                    for r in range(4):
                        dst = xgt_k1[:, 16 * h:16 * (h + 1), 4 * cc + r, :]
                        src = tp2[r * 32:(r + 1) * 32, cc, :].rearrange(
                            "i (t j) -> i t j", j=8
                        )
                        if (cc + r) % 2 == 0:
                            nc.vector.tensor_copy(dst, src)
                        else:
                            nc.scalar.copy(dst, src)

            # single store of both halves -> single writer for the collective
            nc.sync.dma_start(
                xg1_own.rearrange("(h p) n -> p h n", p=128), xg1s[:]
            )

            # ============ AllGather xg1 (single collective) ============
            nc.gpsimd.collective_compute(
                "AllGather",
                mybir.AluOpType.bypass,
                replica_groups=[list(range(NCORE))],
                ins=[xg1_own[:]],
                outs=[xg1_all[:]],
            )

            # ============ stage G2: xg2'[bc, n] = XG1_ALL.T-chunks vs ST ==========
            ga_sb = cpool.tile([128, MCH, BC], BF)
            ga_r = xg1_all.rearrange("(c p) n -> p c n", p=128)
            pst2 = pp.tile([128, 4, NL], DT, tag="g2", name="pst2", bufs=1)
            for mc in range(MCH):
                lde[mc % 2].dma_start(ga_sb[:, mc, :], ga_r[:, mc, :])
                for q in range(4):
                    # start=True zeroes the whole PSUM bank; q pairs
                    # (0,1) and (2,3) share banks, so only q0/q2 start.
                    nc.tensor.matmul(
                        pst2[:, q, :],
                        ga_sb[:, mc, q * 128:(q + 1) * 128],
                        st_sb[:, mc, :],
                        start=(mc == 0 and q % 2 == 0),
                        stop=(mc == MCH - 1),
                        skip_group_check=True,
                    )
            xgt_k2 = xgt_sb[2 * CI:3 * CI, :].rearrange(
                "i (t b j) -> i t b j", b=B, j=8
            )
            for q in range(4):
                for r in range(4):
                    dst = xgt_k2[:, :, 4 * q + r, :]
                    src = pst2[r * 32:(r + 1) * 32, q, :].rearrange(
                        "i (t j) -> i t j", j=8
                    )
                    if (q + r) % 2 == 0:
                        nc.vector.tensor_copy(dst, src)
                    else:
                        nc.scalar.copy(dst, src)

            # ============ stage H: out[b,n,o] = sum_ki xgT[ki,b,n] W[ki,n,o] ======
            # 8 nodes per matmul; psum partition p = b*8+j; full [128,512]
            # psum copied to bf16 staging, diagonal blocks DMA'd per j.
            stg = cpool.tile([128, NT, 512], BF)
            outd_v = outd.rearrange("b (g tp j) o -> b g tp j o", g=8, tp=4, j=8)
            for t in range(NT):
                pso = pp.tile([128, 512], DT, tag="ee", name="pso")
                nc.tensor.matmul(
                    pso[:],
                    xgt_sb[:, t * 128:(t + 1) * 128],
                    wt_sb[:, t * 512:(t + 1) * 512],
                    start=True, stop=True,
                )
                if t % 2 == 1:
                    nc.scalar.copy(stg[:, t, :], pso[:])
                else:
                    nc.vector.tensor_copy(stg[:, t, :], pso[:])
                if t % 4 == 3:
                    g = t // 4
                    for j in range(8):
                        # partitions b*8+j for fixed j: start j, step 8
                        lde[j % 2].dma_start(
                            outd_v[:, g, :, j, :],
                            stg[j::8, 4 * g:4 * (g + 1), 64 * j:64 * (j + 1)],
                        )

            if _DEBUG[0]:
                nc.sync.dma_start(dbg["delt"][:], elt_sb[:])
                for h in range(HN):
                    nc.sync.dma_start(dbg["dS"][h * 128:(h + 1) * 128, :], exps[h][:])
                    nc.sync.dma_start(dbg["dxg1"][h * 128:(h + 1) * 128, :], xg1s[:, h, :])
                nc.sync.dma_start(dbg["dga"][:], ga_sb[:, :].rearrange("p c n -> p (c n)"))
                nc.sync.dma_start(dbg["dxgt"][:], xgt_sb[:])
                nc.sync.dma_start(dbg["dst"][:], st_sb[:, :].rearrange("p c n -> p (c n)"))

    if _SPLIT_WAITS[0]:
        _split_matmul_waits(nc)
    return nc


def _split_matmul_waits(nc):
    """walrus encodes at most one sync-wait per TPB instruction (the EVENTS
    struct has a single wait slot); hoist extra waits onto preceding
    same-engine no-ops."""
    f = nc.m.functions[0]
    for blk in f.blocks:
        insts = blk.instructions
        out = []
        changed = False
        for inst in insts:
            si = inst.sync_info
            if (
                si is not None and si.on_wait and len(si.on_wait) > 1
            ):
                waits = list(si.on_wait)
                for k, w in enumerate(waits[:-1]):
                    nop = mybir.InstNoOp(
                        name=f"{inst.name}-wsplit-{k}",
                        engine=inst.engine,
                        sync_info=mybir.SyncInfo(on_wait=[w], on_update=[]),
                    )
                    out.append(nop)
                inst.sync_info = mybir.SyncInfo(
                    on_wait=[waits[-1]], on_update=list(si.on_update or [])
                )
                changed = True
            out.append(inst)
        if changed:
            blk.instructions = out


def kernel(A, x, conv_E, weight, b):
    A = np.asarray(A, dtype=np.float32)
    x = np.asarray(x, dtype=np.float32)
    conv_E = np.asarray(conv_E, dtype=np.float32)
    weight = np.asarray(weight, dtype=np.float32)
    b = np.asarray(b, dtype=np.float32)

    if "nc" not in _CACHE:
        _CACHE["nc"] = _build()
    nc = _CACHE["nc"]

    import ml_dtypes
    BF = ml_dtypes.bfloat16

    X = np.ascontiguousarray(x.transpose(1, 0, 2).reshape(N, BC))
    X_bf = X.astype(BF)
    ET = np.ascontiguousarray(conv_E.T)
    E_ARR = np.ascontiguousarray(
        conv_E.reshape(MCH, 128, E).transpose(1, 0, 2).reshape(128, MCH * E)
    )
    IDENT = np.eye(128, dtype=BF)
    A1 = A[1]
    # fold Chebyshev combination into weights:
    # out = x@(W0 - W2) + xg1@W1 + (S@xg1)@(2*W2)
    wf = np.empty_like(weight)
    wf[:, 0] = weight[:, 0] - weight[:, 2]
    wf[:, 1] = weight[:, 1]
    wf[:, 2] = 2.0 * weight[:, 2]

    in_maps = []
    for c in range(NCORE):
        sl = slice(c * NL, (c + 1) * NL)
        a1t_c = np.ascontiguousarray(A1[sl, :].T)                       # [N, NL]
        # x slab in [i, t*128 + b*8 + j] layout (n = 8t + j)
        xgt0_c = np.ascontiguousarray(
            X[sl].reshape(NT, 8, B, CI).transpose(3, 0, 2, 1).reshape(CI, NL * B)
        ).astype(BF)
        wt_c = np.ascontiguousarray(
            wf[sl].transpose(1, 2, 0, 3).reshape(KI, NL * CO)
        ).astype(BF)
        in_maps.append({
            "a1t": a1t_c, "et": ET, "e_arr": E_ARR, "xfull": X_bf,
            "xgt0": xgt0_c, "wt": wt_c, "ident": IDENT,
        })

    res = run_bass_kernel_spmd(nc, in_maps, core_ids=list(range(NCORE)), **_RUN_KWARGS)
    _LAST_RESULT[0] = res
    full = np.concatenate(
        [np.asarray(res.results[c]["out"], dtype=np.float32) for c in range(NCORE)],
        axis=1,
    )
    return (full + b[None, None, :]).astype(np.float32)
